# revision 43
# baseline (speedup 1.0000x reference)
"""Trainium2 Bass kernel for nn_ActionDeltaMPredictor.

Pipeline per token: LayerNorm -> SwiGLU (H=1024 hidden) -> Linear to 6x6 ->
skew-symmetrize -> Frobenius clip -> matrix exponential (orthogonal output).

Sharding: pure data parallel over 8 NeuronCores; weights replicated.

Kernel structure per core (B_shard = 16384 tokens = 128 tiles of 128):
  Feeder (per 128-token tile, software-pipelined one group ahead):
    DMA x (fp16), bn_stats LayerNorm stats, rsqrt via bit-trick + Newton
    on DVE (keeps the scalar engine Silu-only -> no ACT_TABLE_LOAD storms),
    normalize to fp16, then an XBAR DMA transpose (dma_start_transpose)
    straight into the feature-major rhs tile -- no PE transposes, no
    PSUM->SBUF copies on the feed path.
  SwiGLU (per 512-token group): gate/val fp16 matmuls at N=512, Silu on
    the scalar engine, h = silu(g)*v on DVE in fp16.
  Head: A_skew = h @ ws where ws = w_out.T - w_out_swapped.T is folded on
    the host (skew-symmetrization is linear), then one DVE copy to fp16
    and an XBAR DMA transpose into token-major [128, 4, 48] (rows padded
    36->48 for the 16-row XBAR tile granularity).
  Expm (per 2048-token batch): Frobenius clip (rsqrt bit-trick), scaling
    and squaring with degree-6 Taylor in fp16 on DVE/GPSIMD, final
    squaring in fp32, DMA out. Ops are interleaved through the next
    batch's groups so neither DVE nor GPSIMD ever blocks the PE feed.
"""

import numpy as np

P = 128
H = 1024
KC = H // P          # contraction chunks
JC = H // P          # hidden chunks
M = 36               # 6x6
MP = 48              # padded slot stride (XBAR tile rows = 16)
GROUP = 4            # token tiles per matmul group
TG = P * GROUP       # tokens per group
LN_EPS = 1e-5
MAX_NORM = 3.0
N_SQ = 2             # squarings (degree-6 Taylor)
INV_SCALE = 1.0 / (1 << N_SQ)
MAGIC = 0x5F3759DF   # rsqrt seed

N_CORES = 8
B_FULL = 131072
B_SHARD = B_FULL // N_CORES


def _fap(t, elem_off, dims):
    """AP over tile t with custom free dims [[step, count], ...]."""
    import concourse.bass as bass
    return bass.AP(
        tensor=t.tensor,
        offset=t.offset + elem_off,
        ap=[list(t.ap[0])] + [list(d) for d in dims],
    )


def build(nc, b_shard, eg=16, has_bias=False, silu_mode='act'):
    """Emit the kernel IR into Bass object nc."""
    import concourse.tile as tile
    import concourse.mybir as mybir

    f32 = mybir.dt.float32
    f16 = mybir.dt.float16
    i32 = mybir.dt.int32
    Alu = mybir.AluOpType
    Act = mybir.ActivationFunctionType

    NT = b_shard // P          # token tiles
    NB = NT // eg              # expm batches
    NG = eg // GROUP           # matmul groups per batch
    NGT = NB * NG              # total groups
    assert NB * eg == NT and NG * GROUP == eg

    x_d = nc.dram_tensor("x", [b_shard, H], f16, kind="ExternalInput")
    wg_d = nc.dram_tensor("wgT", [H, H], f16, kind="ExternalInput")
    wv_d = nc.dram_tensor("wvT", [H, H], f16, kind="ExternalInput")
    ws_d = nc.dram_tensor("wsT", [H, M], f16, kind="ExternalInput")
    if has_bias:
        bg_d = nc.dram_tensor("bg", [H], f32, kind="ExternalInput")
        bv_d = nc.dram_tensor("bv", [H], f32, kind="ExternalInput")
    out_d = nc.dram_tensor("out", [b_shard, M], f32, kind="ExternalOutput")

    GM = eg * MP               # free size of an expm working tile (padded)

    from contextlib import ExitStack
    with tile.TileContext(nc) as tc, ExitStack() as ctx:
        singles = ctx.enter_context(tc.tile_pool(name="singles", bufs=1))
        xp = ctx.enter_context(tc.tile_pool(name="xp", bufs=2))
        statp = ctx.enter_context(tc.tile_pool(name="statp", bufs=2))
        xhp = ctx.enter_context(tc.tile_pool(name="xhp", bufs=2))
        xtap = ctx.enter_context(tc.tile_pool(name="xtap", bufs=2))
        hp = ctx.enter_context(tc.tile_pool(name="hp", bufs=3))
        sgp = ctx.enter_context(tc.tile_pool(name="sgp", bufs=2))
        ap_ = ctx.enter_context(tc.tile_pool(name="ap", bufs=2))
        wp = ctx.enter_context(tc.tile_pool(name="wp", bufs=2))
        ep = ctx.enter_context(tc.tile_pool(name="ep", bufs=1))
        efp = ctx.enter_context(tc.tile_pool(name="efp", bufs=2))
        ps_mm = ctx.enter_context(tc.tile_pool(name="ps_mm", bufs=6, space="PSUM"))
        ps_pa = ctx.enter_context(tc.tile_pool(name="ps_pa", bufs=2, space="PSUM"))

        v = nc.vector
        s = nc.scalar
        gp = nc.gpsimd

        # ---- persistent weights ----
        wg_sb = singles.tile([P, KC, H], f16)
        wv_sb = singles.tile([P, KC, H], f16)
        ws_sb = singles.tile([P, KC, M], f16)

        def load_weights():
            # weights ride the scalar-dispatched DMA queue: the sync queue
            # then serves only x tiles, so the early feeder chains are not
            # serialized behind 4 MB of weights
            nc.scalar.dma_start(wg_sb[:, :, :], wg_d[:, :].rearrange("(c p) j -> p c j", p=P))
            nc.scalar.dma_start(wv_sb[:, :, :], wv_d[:, :].rearrange("(c p) j -> p c j", p=P))
            nc.scalar.dma_start(ws_sb[:, :, :], ws_d[:, :].rearrange("(c p) m -> p c m", p=P))
            if has_bias:
                nc.scalar.dma_start(bg_sb[:, :], bg_d[:].rearrange("(c p) -> p c", p=P))
                nc.scalar.dma_start(bv_sb[:, :], bv_d[:].rearrange("(c p) -> p c", p=P))
        if has_bias:
            bg_sb = singles.tile([P, JC], f32)
            bv_sb = singles.tile([P, JC], f32)
        magic = singles.tile([P, 1], i32)
        v.memset(magic[:, :], MAGIC)

        def rsqrt(y, ve, u, n, iters=2):
            """y <- rsqrt(ve), all [P, n] f32 tiles; u is scratch.

            Bit-trick seed + Newton; keeps sqrt off the scalar engine so it
            never swaps activation tables with Silu.
            """
            yi = y[:, :].bitcast(i32)
            v.tensor_scalar(out=yi, in0=ve[:, :].bitcast(i32), scalar1=1,
                            scalar2=None, op0=Alu.logical_shift_right)
            v.tensor_tensor(out=yi, in0=_fap(magic, 0, [[0, n]]), in1=yi,
                            op=Alu.subtract)
            for _ in range(iters):
                v.tensor_tensor(out=u[:, :], in0=ve[:, :], in1=y[:, :],
                                op=Alu.mult)
                v.tensor_tensor(out=u[:, :], in0=u[:, :], in1=y[:, :],
                                op=Alu.mult)
                v.tensor_scalar(out=u[:, :], in0=u[:, :], scalar1=-0.5,
                                scalar2=1.5, op0=Alu.mult, op1=Alu.add)
                v.tensor_tensor(out=y[:, :], in0=y[:, :], in1=u[:, :],
                                op=Alu.mult)

        def tok(t, off=0, inner=None):
            """Per-token view of an expm tile: eg slots of stride MP."""
            return _fap(t, off, [[MP, eg]] + (inner or [[1, M]]))

        def bmm(ops, dst, a, b, t1, t2, final_add=None):
            """Queue dst = a @ b per 6x6 block over [P, eg, MP] tiles."""
            tmps = [t1, t2]
            for k in range(6):
                aik = _fap(a, k, [[MP, eg], [6, 6], [0, 6]])
                bkj = _fap(b, 6 * k, [[MP, eg], [0, 6], [1, 6]])
                o = dst if k == 0 else tmps[k % 2]
                oij = _fap(o, 0, [[MP, eg], [6, 6], [1, 6]])
                eng = gp if k % 2 == 0 else v
                ops.append(lambda eng=eng, oij=oij, aik=aik, bkj=bkj:
                           eng.tensor_tensor(out=oij, in0=aik, in1=bkj,
                                             op=Alu.mult))
                if k > 0:
                    ops.append(lambda dst=dst, t=tmps[k % 2]:
                               v.tensor_tensor(out=tok(dst), in0=tok(dst),
                                               in1=tok(t), op=Alu.add))
            if final_add is not None:
                ops.append(lambda: v.tensor_tensor(
                    out=tok(dst), in0=tok(dst), in1=tok(final_add),
                    op=Alu.add))

        def queue_expm(W, b):
            """Build the op list for the clip + expm + store of batch b."""
            ops = []
            X = ep.tile([P, GM], f16, tag="X")
            X2 = ep.tile([P, GM], f16, tag="X2")
            X3 = ep.tile([P, GM], f16, tag="X3")
            Q = ep.tile([P, GM], f16, tag="Q")
            E = ep.tile([P, GM], f16, tag="E")
            E2 = ep.tile([P, GM], f16, tag="E2")
            t1 = ep.tile([P, GM], f16, tag="t1")
            t2 = ep.tile([P, GM], f16, tag="t2")
            Ef = efp.tile([P, GM], f32, tag="Ef")
            fr = statp.tile([P, eg], f32, tag="fr")
            yb = statp.tile([P, eg], f32, tag="yb")
            ub = statp.tile([P, eg], f32, tag="ub")
            cs = statp.tile([P, eg], f16, tag="cs")

            def clip_a():
                # squared frobenius norm per token
                v.tensor_tensor(out=tok(t1), in0=tok(W), in1=tok(W),
                                op=Alu.mult)
                v.tensor_reduce(out=_fap(fr, 0, [[1, eg]]),
                                in_=tok(t1),
                                axis=mybir.AxisListType.X, op=Alu.add)
                v.tensor_scalar_add(out=fr[:, :], in0=fr[:, :], scalar1=1e-30)
            ops.append(clip_a)
            ops.append(lambda: rsqrt(yb, fr, ub, eg))

            def clip_b():
                # c = min(frob, 3) * 2^-N_SQ / frob;  frob = fr * y
                v.tensor_tensor(out=ub[:, :], in0=fr[:, :], in1=yb[:, :],
                                op=Alu.mult)
                v.tensor_scalar(out=ub[:, :], in0=ub[:, :], scalar1=MAX_NORM,
                                scalar2=INV_SCALE, op0=Alu.min, op1=Alu.mult)
                v.tensor_tensor(out=cs[:, :], in0=ub[:, :], in1=yb[:, :],
                                op=Alu.mult)
                v.tensor_tensor(out=tok(X), in0=tok(W),
                                in1=_fap(cs, 0, [[1, eg], [0, M]]),
                                op=Alu.mult)
            ops.append(clip_b)
            bmm(ops, X2, X, X, t1, t2)
            bmm(ops, X3, X2, X, t1, t2)

            def poly_setup():
                # E = I + X + X2/2 ; Q = I/6 + X/24 + X2/120 + X3/720
                v.tensor_scalar(out=tok(E), in0=tok(X2), scalar1=0.5,
                                scalar2=None, op0=Alu.mult)
                v.tensor_tensor(out=tok(E), in0=tok(E), in1=tok(X),
                                op=Alu.add)
                ed = _fap(E, 0, [[MP, eg], [7, 6]])
                v.tensor_scalar_add(out=ed, in0=ed, scalar1=1.0)
                v.tensor_scalar(out=tok(Q), in0=tok(X), scalar1=1.0 / 24.0,
                                scalar2=None, op0=Alu.mult)
                v.scalar_tensor_tensor(out=tok(Q), in0=tok(X2),
                                       scalar=1.0 / 120.0, in1=tok(Q),
                                       op0=Alu.mult, op1=Alu.add)
                v.scalar_tensor_tensor(out=tok(Q), in0=tok(X3),
                                       scalar=1.0 / 720.0, in1=tok(Q),
                                       op0=Alu.mult, op1=Alu.add)
                qd = _fap(Q, 0, [[MP, eg], [7, 6]])
                v.tensor_scalar_add(out=qd, in0=qd, scalar1=1.0 / 6.0)
            ops.append(poly_setup)
            bmm(ops, E2, X3, Q, t1, t2, final_add=E)
            assert N_SQ == 2
            bmm(ops, E, E2, E2, t1, t2)
            bmm(ops, Ef, E, E, t1, t2)
            base = b * eg * P
            out_ap = out_d[base:base + eg * P, :].rearrange(
                "(g p) m -> p g m", p=P)
            ops.append(lambda: nc.sync.dma_start(out_ap, tok(Ef)))
            return ops

        pending = []  # expm ops of the previous batch, emitted in chunks

        def emit_pending(frac_done):
            while pending and pending[0][1] <= frac_done:
                pending.pop(0)[0]()

        xtas = {}      # group idx -> xta tile
        W_tiles = {}   # batch idx -> W tile

        def emit_feeder(G):
            """DMA + LN + normalize + XBAR transpose for group G."""
            xta = xtap.tile([P, KC, TG], f16, tag="xta")
            xtas[G] = xta
            mvg = statp.tile([P, GROUP, 2], f32, tag="mvg")
            xts = []
            for sub in range(GROUP):
                ti = G * GROUP + sub
                x_t = xp.tile([P, H], f16, tag=f"x{sub}")
                xts.append(x_t)
                nc.sync.dma_start(x_t[:, :], x_d[ti * P:(ti + 1) * P, :])
                stats = statp.tile([P, 2, 6], f32, tag=f"st{sub}")
                xr = x_t[:, :].rearrange("p (a q) -> p a q", a=2)
                v.bn_stats(out=stats[:, 0, :], in_=xr[:, 0, :])
                v.bn_stats(out=stats[:, 1, :], in_=xr[:, 1, :])
                v.bn_aggr(out=mvg[:, sub, :], in_=stats[:, :, :])
            rst = statp.tile([P, GROUP], f32, tag="rst")
            ve = statp.tile([P, GROUP], f32, tag="ve")
            uu = statp.tile([P, GROUP], f32, tag="uu")
            v.tensor_scalar_add(out=ve[:, :],
                                in0=_fap(mvg, 1, [[2, GROUP]]),
                                scalar1=LN_EPS)
            rsqrt(rst, ve, uu, GROUP)
            for sub in range(GROUP):
                xh = xhp.tile([P, H], f16, tag=f"xh{sub}")
                v.tensor_scalar(out=xh[:, :], in0=xts[sub][:, :],
                                scalar1=mvg[:, sub, 0:1],
                                scalar2=rst[:, sub:sub + 1],
                                op0=Alu.subtract, op1=Alu.mult)
                nc.scalar.dma_start_transpose(
                    xta[:, :, sub * P:(sub + 1) * P], xh[:, :])

        def emit_compute(G):
            """Matmuls + head for group G."""
            b, g = divmod(G, NG)
            xta = xtas.pop(G)
            W = W_tiles[b]
            h = [hp.tile([P, TG], f16, tag=f"h{j0}", name=f"h{j0}")
                 for j0 in range(JC)]
            for j0 in range(JC):
                pg = ps_mm.tile([P, TG], f32, tag="mm")
                for k0 in range(KC):
                    nc.tensor.matmul(
                        pg[:, :],
                        lhsT=wg_sb[:, k0, j0 * P:(j0 + 1) * P],
                        rhs=xta[:, k0, :],
                        start=(k0 == 0), stop=(k0 == KC - 1))
                pv = ps_mm.tile([P, TG], f32, tag="mm")
                for k0 in range(KC):
                    nc.tensor.matmul(
                        pv[:, :],
                        lhsT=wv_sb[:, k0, j0 * P:(j0 + 1) * P],
                        rhs=xta[:, k0, :],
                        start=(k0 == 0), stop=(k0 == KC - 1))
                sg = sgp.tile([P, TG], f32, tag="sg")
                gbias = bg_sb[:, j0:j0 + 1] if has_bias else 0.0
                if silu_mode == 'act':
                    s.activation(out=sg[:, :], in_=pg[:, :], func=Act.Silu,
                                 bias=gbias)
                else:
                    # silu(x) = x * sigmoid(x), for CoreSim (no Silu there)
                    s.activation(out=sg[:, :], in_=pg[:, :],
                                 func=Act.Sigmoid, bias=gbias)
                    if has_bias:
                        v.scalar_tensor_tensor(out=sg[:, :], in0=pg[:, :],
                                               scalar=bg_sb[:, j0:j0 + 1],
                                               in1=sg[:, :],
                                               op0=Alu.add, op1=Alu.mult)
                    else:
                        v.tensor_tensor(out=sg[:, :], in0=sg[:, :],
                                        in1=pg[:, :], op=Alu.mult)
                if has_bias:
                    v.tensor_scalar(out=h[j0][:, :], in0=pv[:, :],
                                    scalar1=bv_sb[:, j0:j0 + 1],
                                    scalar2=None, op0=Alu.add)
                    v.tensor_tensor(out=h[j0][:, :], in0=h[j0][:, :],
                                    in1=sg[:, :], op=Alu.mult)
                else:
                    v.tensor_tensor(out=h[j0][:, :], in0=sg[:, :],
                                    in1=pv[:, :], op=Alu.mult)
            # A_skew = h @ ws (skew fold is in the weights), out [36, TG]
            pa = ps_pa.tile([M, TG], f32, tag="pa")
            for j0 in range(JC):
                nc.tensor.matmul(
                    pa[:, :],
                    lhsT=ws_sb[:, j0, :],
                    rhs=h[j0][:, :],
                    start=(j0 == 0), stop=(j0 == JC - 1))
            a16 = ap_.tile([MP, TG], f16, tag="a16")
            gp.memset(a16[32:MP, :], 0.0)
            v.tensor_copy(out=a16[0:M, :], in_=pa[:, :])
            # token-major [128, GROUP, MP] slice of the batch W tile
            wv_ = _fap(W, g * GROUP * MP, [[MP, GROUP], [1, MP]])
            nc.scalar.dma_start_transpose(wv_, a16[:, :])

        emit_feeder(0)
        load_weights()
        for b in range(NB):
            W_tiles[b] = wp.tile([P, eg * MP], f16, tag="W", name="W")
            for g in range(NG):
                G = b * NG + g
                if G + 1 < NGT:
                    emit_feeder(G + 1)
                emit_pending((g + 1) / NG)
                emit_compute(G)
            while pending:
                pending.pop(0)[0]()
            ops = queue_expm(W_tiles.pop(b), b)
            n = len(ops)
            pending = [(op, 0.85 * (i + 1) / n) for i, op in enumerate(ops)]
        while pending:
            pending.pop(0)[0]()

    return nc


def split_waits(nc, ctrl_limit=1, limit=1):
    """Walrus codegen caps sem-wait commands per instruction (1 for CTRL-type
    Drain/NoOp). Hoist excess waits onto preceding same-engine NoOps."""
    import concourse.mybir as mybir
    n = 0
    for fn in nc.m.functions:
        for blk in fn.blocks:
            out = []
            changed = False
            for inst in blk.instructions:
                lim = (ctrl_limit if inst.opcode in
                       ("Drain", "NoOp", "EventSemaphoreOp") else limit)
                si = inst.sync_info
                waits = list(si.on_wait) if si is not None and si.on_wait else []
                if len(waits) > lim:
                    head, tail = waits[:-lim], waits[-lim:]
                    k = 0
                    while head:
                        chunk, head = head[:ctrl_limit], head[ctrl_limit:]
                        out.append(mybir.InstNoOp(
                            name=f"{inst.name}-wsplit{k}",
                            engine=inst.engine, ins=[], outs=[],
                            sync_info=mybir.SyncInfo(on_wait=chunk, on_update=[]),
                        ))
                        k += 1
                        n += 1
                    si.on_wait = tail
                    changed = True
                out.append(inst)
            if changed:
                blk.instructions = out
    return n


def _build_program(b_shard, eg, has_bias):
    import concourse.bass as bass
    nc = bass.Bass()
    build(nc, b_shard, eg=eg, has_bias=has_bias)
    split_waits(nc)
    return nc


def prepare_inputs(state_token, gamma, beta, w_gate, w_val, w_out):
    """Host-side preprocessing: fold gamma/beta into weights, fold the
    skew-symmetrization into w_out, cast to fp16."""
    gamma = np.asarray(gamma, np.float32)
    beta = np.asarray(beta, np.float32)
    w_gate = np.asarray(w_gate, np.float32)
    w_val = np.asarray(w_val, np.float32)
    w_out = np.asarray(w_out, np.float32)
    wgT = np.ascontiguousarray((w_gate * gamma).T).astype(np.float16)
    wvT = np.ascontiguousarray((w_val * gamma).T).astype(np.float16)
    # skew fold: As[t,(i,j)] = sum_k h[t,k] * (w_out[(i,j),k] - w_out[(j,i),k])
    w_o66 = w_out.reshape(6, 6, H)
    ws = (w_o66 - np.swapaxes(w_o66, 0, 1)).reshape(M, H)
    wsT = np.ascontiguousarray(ws.T).astype(np.float16)
    bg = w_gate @ beta
    bv = w_val @ beta
    has_bias = bool(np.any(bg) or np.any(bv))
    return wgT, wvT, wsT, bg, bv, has_bias


def kernel(state_token, gamma, beta, w_gate, w_val, w_out):
    from concourse.bass_utils import run_bass_kernel_spmd

    x = np.asarray(state_token, np.float32)
    assert x.shape == (B_FULL, H), x.shape
    x16 = x.astype(np.float16)
    wgT, wvT, wsT, bg, bv, has_bias = prepare_inputs(
        state_token, gamma, beta, w_gate, w_val, w_out)

    nc = _build_program(B_SHARD, 16, has_bias)

    in_maps = []
    for i in range(N_CORES):
        m = {
            "x": np.ascontiguousarray(x16[i * B_SHARD:(i + 1) * B_SHARD]),
            "wgT": wgT, "wvT": wvT, "wsT": wsT,
        }
        if has_bias:
            m["bg"] = bg
            m["bv"] = bv
        in_maps.append(m)

    import os
    tmpdir = os.environ.get("KERNEL_TRACE_DIR") or None
    res = run_bass_kernel_spmd(nc, in_maps, list(range(N_CORES)), tmpdir=tmpdir)
    kernel._last = res  # for test harness introspection (exec time, trace)
    out = np.concatenate([res.results[i]["out"] for i in range(N_CORES)], axis=0)
    return out.reshape(B_FULL, 6, 6)
